# revision 1
# baseline (speedup 1.0000x reference)
"""Trainium2 Bass kernel for a pre-norm transformer block (B=4, N=1024, C=1024,
16 heads, MLP hidden 4096), SPMD across 8 NeuronCores.

Sharding: core = (b, s) with b = batch element (4), s = query-half (2).
Each core computes the block for 512 query tokens of one batch element:
  - LN1 + K/V projections over the full 1024-token sequence of its batch
    element (duplicated within the batch pair - avoids all collectives),
  - Q projection + attention + proj + residual + LN2 + MLP for its 512 rows.

Everything on-device is channel-major (channels on partitions, tokens on the
free dim) so no on-device transposes are needed; the host passes x.T and
pre-transposed weights. LayerNorm gains/biases are folded into the following
matmul weights/biases on the host; the softmax scale is folded into the Q
weights. Softmax uses no max-subtraction (logits are O(1) by construction),
which makes the softmax1 "+1" denominator term exact and free; the
denominator itself comes from a ones-column appended to V inside the P@V
matmul.

Precision: main matmuls in float32r (full PE rate, ~1e-4 matmul rel err);
attention S/P/O and the MLP hidden activations in bf16; everything else fp32.
"""

from contextlib import ExitStack

import numpy as np
import ml_dtypes

import concourse.bass as bass
import concourse.mybir as mybir
import concourse.tile as tile
from concourse import bacc

P = 128
B, N, C = 4, 1024, 1024
H, D = 16, 64
HD = 4 * C
NQ = N // 2          # query tokens per core
NK = N               # key/value tokens per core
CT = C // P          # 8 channel tiles
KB = NK // P         # 8 key-token tiles
SCALE = D ** (-0.5)
EPS = 1e-5

F32 = mybir.dt.float32
F32R = mybir.dt.float32r
BF16 = mybir.dt.bfloat16
AF = mybir.ActivationFunctionType
ALU = mybir.AluOpType


def _pb(ap, p):
    """Partition-broadcast: view a [1, ...] AP as [p, ...] with 0 partition
    stride (legal for DMA reads)."""
    return bass.AP(tensor=ap.tensor, offset=ap.offset,
                   ap=[[0, p]] + [list(x) for x in ap.ap[1:]])


def build_nc(stop_after=None):
    nc = _build_body(stop_after)
    nc.compile()
    return nc


def _build_body(stop_after=None):
    nc = bacc.Bacc("TRN2", target_bir_lowering=False, debug=False, num_devices=8)

    # ---- DRAM I/O (per core) ----
    xT = nc.dram_tensor("xT", [C, NK], F32R, kind="ExternalInput")
    ones_d = nc.dram_tensor("ones_d", [P], F32R, kind="ExternalInput")
    xTqb = nc.dram_tensor("xTqb", [C, NQ], F32, kind="ExternalInput")      # x.T[:, q] + proj_b
    maskT = nc.dram_tensor("maskT", [NK, NQ], BF16, kind="ExternalInput")  # 1-mask, transposed
    qkv_wT = nc.dram_tensor("qkv_wT", [C, 3 * C], BF16, kind="ExternalInput")
    qkvb_qk = nc.dram_tensor("qkvb_qk", [2 * C], F32, kind="ExternalInput")
    qkvb_v = nc.dram_tensor("qkvb_v", [1, C], BF16, kind="ExternalInput")
    proj_wT = nc.dram_tensor("proj_wT", [C, C], BF16, kind="ExternalInput")
    fc1_wT = nc.dram_tensor("fc1_wT", [C, HD], BF16, kind="ExternalInput")
    fc1b = nc.dram_tensor("fc1b", [HD], F32, kind="ExternalInput")
    fc2_wT = nc.dram_tensor("fc2_wT", [HD, C], BF16, kind="ExternalInput")
    fc2b = nc.dram_tensor("fc2b", [C], F32, kind="ExternalInput")
    outT = nc.dram_tensor("outT", [C, NQ], F32, kind="ExternalOutput")

    with tile.TileContext(nc) as tc, ExitStack() as ctx:
        persist = ctx.enter_context(tc.tile_pool(name="persist", bufs=1))
        hpool = ctx.enter_context(tc.tile_pool(name="hpool", bufs=1))
        xstream = ctx.enter_context(tc.tile_pool(name="xstream", bufs=4))
        sqpool = ctx.enter_context(tc.tile_pool(name="sqpool", bufs=2))
        wpool = ctx.enter_context(tc.tile_pool(name="wpool", bufs=2))
        vwpool = ctx.enter_context(tc.tile_pool(name="vwpool", bufs=4))
        wpool2 = ctx.enter_context(tc.tile_pool(name="wpool2", bufs=3))
        wpool3 = ctx.enter_context(tc.tile_pool(name="wpool3", bufs=3))
        ptpool = ctx.enter_context(tc.tile_pool(name="ptpool", bufs=4))
        rows2 = ctx.enter_context(tc.tile_pool(name="rows2", bufs=3))
        bcast = ctx.enter_context(tc.tile_pool(name="bcast", bufs=4))
        bcast2 = ctx.enter_context(tc.tile_pool(name="bcast2", bufs=2))
        rbpool = ctx.enter_context(tc.tile_pool(name="rbpool", bufs=2))
        outpool = ctx.enter_context(tc.tile_pool(name="outpool", bufs=2))
        xqpool = ctx.enter_context(tc.tile_pool(name="xqpool", bufs=2))
        mpool = ctx.enter_context(tc.tile_pool(name="mpool", bufs=3))
        mstream = ctx.enter_context(tc.tile_pool(name="mstream", bufs=2))
        mdram = ctx.enter_context(tc.tile_pool(name="mdram", bufs=1, space="DRAM"))

        # ---------- constants / biases ----------
        ones_col = persist.tile([P, 1], F32R, tag="ones_col")
        nc.sync.dma_start(ones_col, ones_d.rearrange("(p o) -> p o", o=1))
        ones_k1 = persist.tile([1, P], BF16, tag="ones_k1")
        nc.gpsimd.dma_start(ones_k1, ones_d.rearrange("(o p) -> o p", o=1))
        epst = persist.tile([1, 1], F32, tag="eps")
        nc.vector.memset(epst, EPS)
        qkb_sb = persist.tile([P, 16], F32, tag="qkb")
        nc.sync.dma_start(qkb_sb, qkvb_qk.rearrange("(o p) -> p o", p=P))
        fc1b_sb = persist.tile([P, 32], F32, tag="fc1b")
        nc.sync.dma_start(fc1b_sb, fc1b.rearrange("(o p) -> p o", p=P))
        fc2b_sb = persist.tile([P, 8], F32, tag="fc2b")
        nc.sync.dma_start(fc2b_sb, fc2b.rearrange("(o p) -> p o", p=P))
        vb_row = persist.tile([1, C], BF16, tag="vb")
        nc.sync.dma_start(vb_row, qkvb_v.ap())
        ones_k1f = persist.tile([1, P], F32R, tag="ones_k1f")
        nc.sync.dma_start(ones_k1f, ones_d.rearrange("(o p) -> o p", o=1))

        # mask (bf16, [k, q] as [128, kb, q])
        mask_sb = persist.tile([P, KB, NQ], BF16, tag="mask")
        mT = maskT.rearrange("(kb p) q -> p kb q", p=P)
        nc.sync.dma_start(mask_sb, mT)

        # ---------- phases 1+2: LN1, pipelined per token-half ----------
        # hT is split into two tiles (one per 512-token half) so the QKV
        # matmuls for half 0 can start while half 1 is still normalizing.
        xTr = xT.rearrange("(ct p) n -> p ct n", p=P)
        hTh = []
        psA_ctx = ExitStack()
        psA = psA_ctx.enter_context(tc.tile_pool(name="psA", bufs=2, space="PSUM"))
        for nh in range(2):
            sl = slice(nh * 512, (nh + 1) * 512)
            ps_s1 = psA.tile([1, 512], F32, tag="s", name=f"ps_s1_{nh}")
            ps_s2 = psA.tile([1, 512], F32, tag="s2", name=f"ps_s2_{nh}")
            for t in range(CT // 2):
                xc = xstream.tile([P, 2, 512], F32R, tag="xc", name=f"xc_{nh}_{t}")
                nc.sync.dma_start(xc, xTr[:, 2 * t:2 * t + 2, sl])
                for i in range(2):
                    ct = 2 * t + i
                    sq = sqpool.tile([P, 512], F32R, tag="sq", name=f"sq_{nh}_{ct}")
                    nc.scalar.activation(sq, xc[:, i, :].bitcast(F32), AF.Square)
                    nc.tensor.matmul(ps_s1, ones_col, xc[:, i, :],
                                     start=(ct == 0), stop=(ct == CT - 1))
                    nc.tensor.matmul(ps_s2, ones_col, sq,
                                     start=(ct == 0), stop=(ct == CT - 1))
            mu_row = rows2.tile([1, 512], F32, tag="r512ln", name=f"mu_{nh}")
            nc.vector.tensor_scalar_mul(mu_row, ps_s1, 1.0 / C)
            e2_row = rows2.tile([1, 512], F32, tag="r512ln", name=f"e2_{nh}")
            nc.vector.tensor_scalar_mul(e2_row, ps_s2, 1.0 / C)
            tmp_row = rows2.tile([1, 512], F32, tag="r512ln", name=f"tmp_{nh}")
            nc.vector.tensor_tensor(tmp_row, mu_row, mu_row, ALU.mult)
            nc.vector.tensor_tensor(e2_row, e2_row, tmp_row, ALU.subtract)
            nc.scalar.activation(e2_row, e2_row, AF.Sqrt, bias=epst[:, :])
            nc.vector.reciprocal(e2_row, e2_row)
            mu_bc = bcast.tile([P, 512], F32, tag="b512", name=f"mu_b_{nh}")
            nc.gpsimd.partition_broadcast(mu_bc[:, :], mu_row[:, :])
            rs_bc = bcast.tile([P, 512], F32, tag="b512", name=f"rs_b_{nh}")
            nc.gpsimd.partition_broadcast(rs_bc[:, :], e2_row[:, :])
            hT = hpool.tile([P, CT, 512], BF16, tag=f"h{nh}")
            tmpf = hpool.tile([P, 512], F32, tag=f"htmp{nh}")
            for t in range(CT // 2):
                xc = xstream.tile([P, 2, 512], F32R, tag="xc", name=f"xc2_{nh}_{t}")
                nc.sync.dma_start(xc, xTr[:, 2 * t:2 * t + 2, sl])
                for i in range(2):
                    ct = 2 * t + i
                    nc.vector.tensor_tensor(tmpf, xc[:, i, :].bitcast(F32), mu_bc,
                                            ALU.subtract)
                    nc.vector.tensor_tensor(hT[:, ct, :], tmpf, rs_bc, ALU.mult)
            hTh.append(hT)
        psA_ctx.close()

        if stop_after == 'ln1':
            return nc
        # ---------- phase 3: q^T, k^T (channel-major, f32r matmul) ----------
        # Host passes xT ROLLED per core: the query half is always tokens
        # [0:512] (= hTh[0]); K/V cover both halves with mask columns matched.
        qT = persist.tile([P, CT, NQ], BF16, tag="qT")
        kT = persist.tile([P, CT, NK], BF16, tag="kT")
        wT = qkv_wT.rearrange("(ct p) m -> p ct m", p=P)
        psB_ctx = ExitStack()
        psB = psB_ctx.enter_context(tc.tile_pool(name="psB", bufs=4, space="PSUM"))
        for wg in range(4):          # 4 groups of 4 chan-blocks (q,q,k,k)
            is_q = wg < 2
            nhs = [0] if is_q else [0, 1]
            for nh in nhs:
                tsl = slice(nh * 512, (nh + 1) * 512)
                pss = [psB.tile([P, 512], F32, tag="mm", name=f"ps_qk{wg}_{nh}_{j}")
                       for j in range(4)]
                for t in range(CT // 2):
                    wt = wpool.tile([P, 2, 512], BF16, tag="w")
                    nc.sync.dma_start(wt, wT[:, 2 * t:2 * t + 2, wg * 512:(wg + 1) * 512])
                    for i in range(2):
                        ct = 2 * t + i
                        for j in range(4):
                            nc.tensor.matmul(pss[j], wt[:, i, j * P:(j + 1) * P],
                                             hTh[nh][:, ct, :],
                                             start=(ct == 0), stop=(ct == CT - 1))
                for j in range(4):
                    mb = wg * 4 + j
                    bias = qkb_sb[:, mb:mb + 1]
                    if is_q:
                        nc.scalar.activation(qT[:, mb, :], pss[j], AF.Identity, bias=bias)
                    else:
                        nc.scalar.activation(kT[:, mb - 8, tsl], pss[j], AF.Identity,
                                             bias=bias)

        if stop_after == 'qk':
            psB_ctx.close()
            return nc
        # ---------- phase 4: v token-major + ones column ----------
        # V_aug: [128 tok, kb, head, 65] (65th col = 1.0 for denominator)
        V_aug = persist.tile([P, KB, H, 65], BF16, tag="vaug")
        nc.vector.memset(V_aug[:, :, :, 64:65], 1.0)
        psV_ctx = ExitStack()
        psV = psV_ctx.enter_context(tc.tile_pool(name="psV", bufs=4, space="PSUM"))
        for nh in range(4):
            wvs = []
            for t in range(CT // 2):
                wv = vwpool.tile([P, 2, 256], BF16, tag="vw", name=f"vw_{nh}_{t}")
                nc.sync.dma_start(wv, wT[:, 2 * t:2 * t + 2,
                                         2048 + nh * 256: 2048 + (nh + 1) * 256])
                wvs.append(wv)
            for tb in range(KB):
                hsrc = hTh[tb // 4][:, :, (tb % 4) * P:(tb % 4 + 1) * P]
                psv = psV.tile([P, 256], F32, tag="mmv", name=f"psv_{nh}_{tb}")
                for ct in range(CT):
                    nc.tensor.matmul(psv, hsrc[:, ct, :],
                                     wvs[ct // 2][:, ct % 2, :],
                                     start=(ct == 0), stop=False)
                nc.tensor.matmul(psv, ones_k1,
                                 vb_row[:, nh * 256:(nh + 1) * 256],
                                 start=False, stop=True)
                nc.scalar.activation(
                    V_aug[:, tb, nh * 4:(nh + 1) * 4, 0:64],
                    psv.rearrange("p (h d) -> p h d", d=64),
                    AF.Copy)
        psV_ctx.close()
        psB_ctx.close()

        if stop_after == 'v':
            return nc
        # ---------- phase 5: attention per head ----------
        # S psums in [128, 2, 512] groups -> 1024-wide exp and mask ops.
        # Mask-multiply alternates DVE / GPSIMD to split the elementwise load.
        psS_ctx = ExitStack()
        psS = psS_ctx.enter_context(tc.tile_pool(name="psS", bufs=2, space="PSUM"))
        psO = psS_ctx.enter_context(tc.tile_pool(name="psO", bufs=2, space="PSUM"))
        oT = persist.tile([P, CT, NQ], BF16, tag="oT")
        for h in range(H):
            j, base = h // 2, (h % 2) * 64
            po = psO.tile([65, NQ], F32, tag="o")
            for g in range(4):
                psg = psS.tile([P, 2, 512], F32, tag="sg", name=f"psg_{h}_{g}")
                for i in range(2):
                    kb = 2 * g + i
                    nc.tensor.matmul(psg[:, i, :],
                                     kT[base:base + 64, j, kb * P:(kb + 1) * P],
                                     qT[base:base + 64, j, :], start=True, stop=True)
                PT = ptpool.tile([P, 2, NQ], BF16, tag="pt", name=f"pt_{h}_{g}")
                nc.scalar.activation(PT, psg, AF.Exp)
                nc.vector.tensor_tensor(PT, PT, mask_sb[:, 2 * g:2 * g + 2, :], ALU.mult)
                for i in range(2):
                    kb = 2 * g + i
                    nc.tensor.matmul(po, V_aug[:, kb, h, :], PT[:, i, :],
                                     start=(kb == 0), stop=(kb == KB - 1))
            drow = rows2.tile([1, NQ], F32, tag="r512")
            nc.vector.tensor_scalar_add(drow, po[64:65, :], 1.0)
            nc.vector.reciprocal(drow, drow)
            rb = rbpool.tile([64, NQ], F32, tag="rb")
            nc.gpsimd.partition_broadcast(rb[:, :], drow[:, :])
            nc.vector.tensor_tensor(oT[base:base + 64, j, :], po[0:64, :], rb, ALU.mult)
        psS_ctx.close()

        if stop_after == 'attn':
            return nc
        # ---------- phase 6: proj + residual -> x2T ----------
        psB2_ctx = ExitStack()
        psB2 = psB2_ctx.enter_context(tc.tile_pool(name="psB2", bufs=4, space="PSUM"))
        x2T = persist.tile([P, CT, NQ], F32R, tag="x2T")
        pTr = proj_wT.rearrange("(ct p) m -> p ct m", p=P)
        for ob in range(CT):
            wp = wpool3.tile([P, CT, P], BF16, tag="wp")
            nc.sync.dma_start(wp, pTr[:, :, ob * P:(ob + 1) * P])
            psp = psB2.tile([P, 512], F32, tag="mm")
            for ct in range(CT):
                nc.tensor.matmul(psp, wp[:, ct], oT[:, ct, :],
                                 start=(ct == 0), stop=(ct == CT - 1))
            xq = xqpool.tile([P, NQ], F32, tag="xq")
            nc.sync.dma_start(xq, xTqb[ob * P:(ob + 1) * P, :])
            nc.vector.tensor_tensor(x2T[:, ob, :], psp, xq, ALU.add)

        if stop_after == 'proj':
            psB2_ctx.close()
            return nc
        # ---------- phase 7: LN2 ----------
        psC_ctx = ExitStack()
        psC = psC_ctx.enter_context(tc.tile_pool(name="psC", bufs=2, space="PSUM"))
        ps_t1 = psC.tile([1, NQ], F32, tag="s2")
        ps_t2 = psC.tile([1, NQ], F32, tag="s2")
        for ob in range(CT):
            sq2 = sqpool.tile([P, 512], F32R, tag="sq")
            nc.scalar.activation(sq2, x2T[:, ob, :].bitcast(F32), AF.Square)
            nc.tensor.matmul(ps_t1, ones_col, x2T[:, ob, :],
                             start=(ob == 0), stop=(ob == CT - 1))
            nc.tensor.matmul(ps_t2, ones_col, sq2,
                             start=(ob == 0), stop=(ob == CT - 1))
        mu2 = rows2.tile([1, NQ], F32R, tag="r512ln")
        nc.vector.tensor_scalar_mul(mu2, ps_t1, 1.0 / C)
        e22 = rows2.tile([1, NQ], F32R, tag="r512ln")
        nc.vector.tensor_scalar_mul(e22, ps_t2, 1.0 / C)
        tmp2 = rows2.tile([1, NQ], F32R, tag="r512ln")
        nc.vector.tensor_tensor(tmp2, mu2.bitcast(F32), mu2.bitcast(F32), ALU.mult)
        nc.vector.tensor_tensor(e22, e22.bitcast(F32), tmp2.bitcast(F32), ALU.subtract)
        nc.scalar.activation(e22, e22.bitcast(F32), AF.Sqrt, bias=epst[:, :])
        with nc.allow_low_precision(reason="f32r rsig rounding ~1e-4 is fine"):
            nc.vector.reciprocal(e22, e22.bitcast(F32))
        rs2 = e22
        mu2b = psC.tile([P, NQ], F32, tag="bc2")
        nc.tensor.matmul(mu2b, ones_k1f, mu2, start=True, stop=True)
        rs2b = psC.tile([P, NQ], F32, tag="bc2")
        nc.tensor.matmul(rs2b, ones_k1f, rs2, start=True, stop=True)
        h2T = hpool.tile([P, CT, NQ], BF16, tag="h0")
        h2tmp = hpool.tile([P, 512], F32, tag="htmp0")
        for ob in range(CT):
            nc.vector.tensor_tensor(h2tmp, x2T[:, ob, :].bitcast(F32), mu2b,
                                    ALU.subtract)
            nc.vector.tensor_tensor(h2T[:, ob, :], h2tmp, rs2b, ALU.mult)
        psC_ctx.close()

        if stop_after == 'ln2':
            psB2_ctx.close()
            return nc
        # ---------- phase 8a: fc1 + gelu -> m (bf16, spilled to DRAM) ----------
        m_dram = mdram.tile([32, P, NQ], BF16)
        w1T = fc1_wT.rearrange("(ct p) m -> p ct m", p=P)
        for hg in range(8):
            pss = [psB2.tile([P, 512], F32, tag="mm", name=f"ps_fc1_{hg}_{j}")
                   for j in range(4)]
            w1 = wpool.tile([P, CT, 512], BF16, tag="w1")
            nc.sync.dma_start(w1, w1T[:, :, hg * 512:(hg + 1) * 512])
            for ct in range(CT):
                for j in range(4):
                    nc.tensor.matmul(pss[j], w1[:, ct, j * P:(j + 1) * P],
                                     h2T[:, ct, :],
                                     start=(ct == 0), stop=(ct == CT - 1))
            for j in range(4):
                hb = hg * 4 + j
                mo = mpool.tile([P, NQ], BF16, tag="mo")
                nc.scalar.activation(mo, pss[j], AF.Gelu, bias=fc1b_sb[:, hb:hb + 1])
                nc.sync.dma_start(m_dram[hb], mo)
        psB2_ctx.close()

        if stop_after == 'fc1':
            return nc
        # ---------- phase 8b: fc2 + bias + residual -> out ----------
        psD_ctx = ExitStack()
        psD = psD_ctx.enter_context(tc.tile_pool(name="psD", bufs=8, space="PSUM"))
        ps_oc = [psD.tile([P, 512], F32, tag="fc2", name=f"ps_fc2_{ob}")
                 for ob in range(8)]
        w2T = fc2_wT.rearrange("(ht p) m -> p ht m", p=P)
        for tp in range(16):
            w2 = wpool2.tile([P, 2, C], BF16, tag="w2")
            nc.sync.dma_start(w2, w2T[:, 2 * tp:2 * tp + 2, :])
            mi = mstream.tile([P, 2, NQ], BF16, tag="mi")
            nc.sync.dma_start(mi, m_dram[2 * tp:2 * tp + 2].rearrange("h p q -> p h q"))
            for i in range(2):
                ht = 2 * tp + i
                for ob in range(CT):
                    nc.tensor.matmul(ps_oc[ob], w2[:, i, ob * P:(ob + 1) * P],
                                     mi[:, i, :],
                                     start=(ht == 0), stop=(ht == 31))
        for ob in range(CT):
            ot = outpool.tile([P, NQ], F32, tag="out")
            nc.vector.tensor_scalar(ot, ps_oc[ob], fc2b_sb[:, ob:ob + 1], None, ALU.add)
            nc.vector.tensor_tensor(ot, ot, x2T[:, ob, :].bitcast(F32), ALU.add)
            nc.sync.dma_start(outT[ob * P:(ob + 1) * P, :], ot)
        psD_ctx.close()

    return nc


# ---------------------------------------------------------------------------
# Host side: shard, run, gather
# ---------------------------------------------------------------------------
_RUNNER = None


class _Runner:
    """Minimal SPMD executor via bass2jax custom call (axon PJRT path)."""

    def __init__(self, nc, n_cores):
        import jax
        from jax.sharding import Mesh, PartitionSpec
        from jax.experimental.shard_map import shard_map
        from concourse.bass2jax import (_bass_exec_p, install_neuronx_cc_hook,
                                        partition_id_tensor)
        install_neuronx_cc_hook()
        self.jax = jax
        self.nc = nc
        self.n_cores = n_cores
        partition_name = nc.partition_id_tensor.name if nc.partition_id_tensor else None
        in_names, out_names, out_avals, zero_outs = [], [], [], []
        for alloc in nc.m.functions[0].allocations:
            if not isinstance(alloc, mybir.MemoryLocationSet):
                continue
            name = alloc.memorylocations[0].name
            if alloc.kind == "ExternalInput":
                if name != partition_name:
                    in_names.append(name)
            elif alloc.kind == "ExternalOutput":
                shape = tuple(alloc.tensor_shape)
                dtype = mybir.dt.np(alloc.dtype)
                out_names.append(name)
                out_avals.append(jax.core.ShapedArray(shape, dtype))
                zero_outs.append(np.zeros(shape, dtype))
        self.in_names, self.out_names = in_names, out_names
        self._out_avals, self._zero_outs = out_avals, zero_outs
        n_params = len(in_names)
        all_in = in_names + out_names + ([partition_name] if partition_name else [])

        def _body(*args):
            operands = list(args)
            if partition_name is not None:
                operands.append(partition_id_tensor())
            return tuple(_bass_exec_p.bind(
                *operands, out_avals=tuple(out_avals), in_names=tuple(all_in),
                out_names=tuple(out_names), lowering_input_output_aliases=(),
                sim_require_finite=True, sim_require_nnan=True, nc=nc))

        devices = jax.devices()[:n_cores]
        mesh = Mesh(np.asarray(devices), ("core",))
        nspec = n_params + len(out_names)
        self._fn = jax.jit(
            shard_map(_body, mesh=mesh, in_specs=(PartitionSpec("core"),) * nspec,
                      out_specs=(PartitionSpec("core"),) * len(out_names),
                      check_rep=False),
            keep_unused=True)

    def run(self, in_maps):
        n = self.n_cores
        per_core = [[np.ascontiguousarray(m[k]) for k in self.in_names] for m in in_maps]
        args = [np.concatenate([per_core[c][i] for c in range(n)], axis=0)
                for i in range(len(self.in_names))]
        args += [np.zeros((n * z.shape[0], *z.shape[1:]), z.dtype) for z in self._zero_outs]
        outs = self._fn(*args)
        self.jax.block_until_ready(outs)
        return [
            {name: np.asarray(outs[i]).reshape(n, *self._out_avals[i].shape)[c]
             for i, name in enumerate(self.out_names)}
            for c in range(n)
        ]


def _get_runner():
    global _RUNNER
    if _RUNNER is None:
        _RUNNER = _Runner(build_nc(), 8)
    return _RUNNER


def kernel(x, mask, ln1_g, ln1_b, qkv_w, qkv_b, proj_w, proj_b,
           ln2_g, ln2_b, fc1_w, fc1_b, fc2_w, fc2_b):
    x = np.asarray(x, np.float32)
    mask = np.asarray(mask, bool)
    ln1_g = np.asarray(ln1_g, np.float32); ln1_b = np.asarray(ln1_b, np.float32)
    qkv_w = np.asarray(qkv_w, np.float32); qkv_b = np.asarray(qkv_b, np.float32)
    proj_w = np.asarray(proj_w, np.float32); proj_b = np.asarray(proj_b, np.float32)
    ln2_g = np.asarray(ln2_g, np.float32); ln2_b = np.asarray(ln2_b, np.float32)
    fc1_w = np.asarray(fc1_w, np.float32); fc1_b = np.asarray(fc1_b, np.float32)
    fc2_w = np.asarray(fc2_w, np.float32); fc2_b = np.asarray(fc2_b, np.float32)

    # fold LN1 gain/bias + softmax scale into qkv weights/bias
    Wq = qkv_w * ln1_g[None, :]
    qb = qkv_w @ ln1_b + qkv_b
    Wq[:C] *= SCALE
    qb = qb.copy(); qb[:C] *= SCALE
    qkv_wT = np.ascontiguousarray(Wq.T).astype(ml_dtypes.bfloat16)
    # fold LN2 gain/bias into fc1
    W1 = fc1_w * ln2_g[None, :]
    fb1 = fc1_w @ ln2_b + fc1_b
    fc1_wT = np.ascontiguousarray(W1.T).astype(ml_dtypes.bfloat16)
    proj_wT = np.ascontiguousarray(proj_w.T).astype(ml_dtypes.bfloat16)
    fc2_wT = np.ascontiguousarray(fc2_w.T).astype(ml_dtypes.bfloat16)
    qkvb_v = np.ascontiguousarray(qb[2 * C:].reshape(1, C)).astype(ml_dtypes.bfloat16)
    maskmul = (~mask).astype(ml_dtypes.bfloat16)

    in_maps = []
    for core in range(8):
        b, s = core // 2, core % 2
        # roll x so this core's query half is always tokens [0:512]
        xb = x[b]                         # [N, C]
        xroll = np.roll(xb, -s * NQ, axis=0)
        xT_c = np.ascontiguousarray(xroll.T)               # [C, NK]
        xTqb_c = np.ascontiguousarray(xb[s * NQ:(s + 1) * NQ].T) + proj_b[:, None]
        # mask rows q = this core's queries; key order must match rolled order
        mrow = maskmul[b, s * NQ:(s + 1) * NQ]             # [NQ, N] keys orig order
        mroll = np.roll(mrow, -s * NQ, axis=1)             # keys in rolled order
        maskT_c = np.ascontiguousarray(mroll.T)            # [NK, NQ]
        in_maps.append({
            "ones_d": np.ones(P, np.float32),
            "xT": xT_c.astype(np.float32),
            "xTqb": xTqb_c.astype(np.float32),
            "maskT": maskT_c,
            "qkv_wT": qkv_wT, "qkvb_qk": qb[:2 * C].copy(), "qkvb_v": qkvb_v,
            "proj_wT": proj_wT,
            "fc1_wT": fc1_wT, "fc1b": fb1.copy(),
            "fc2_wT": fc2_wT, "fc2b": fc2_b.copy(),
        })

    results = _get_runner().run(in_maps)
    out = np.empty((B, N, C), np.float32)
    for core in range(8):
        b, s = core // 2, core % 2
        out[b, s * NQ:(s + 1) * NQ, :] = results[core]["outT"].T
    return out



# revision 2
# speedup vs baseline: 1.2443x; 1.2443x over previous
"""Trainium2 Bass kernel for a pre-norm transformer block (B=4, N=1024, C=1024,
16 heads, MLP hidden 4096), SPMD across 8 NeuronCores.

Sharding: core = (b, s) with b = batch element (4), s = query-half (2).
Each core computes the block for 512 query tokens of one batch element:
  - LN1 + K/V projections over the full 1024-token sequence of its batch
    element (duplicated within the batch pair - avoids all collectives),
  - Q projection + attention + proj + residual + LN2 + MLP for its 512 rows.

Everything on-device is channel-major (channels on partitions, tokens on the
free dim) so no on-device transposes are needed; the host passes x.T and
pre-transposed weights. LayerNorm gains/biases are folded into the following
matmul weights/biases on the host; the softmax scale is folded into the Q
weights. Softmax uses no max-subtraction (logits are O(1) by construction),
which makes the softmax1 "+1" denominator term exact and free; the
denominator itself comes from a ones-column appended to V inside the P@V
matmul.

Precision: QKV / V / proj / fc1 / fc2 matmuls run in fp8 (e4m3) with
DoubleRow perf mode (2 contraction tiles per instruction, 0.5 PE cycles per
output column). Weights are scaled by 1024 on the host (dequantized for free
via the activation-engine `scale` on the PSUM->SBUF copies); activations are
quantized at scale 1 (h, h2, m) or 64 (o).  fc1 uses a two-term fp8 split of
BOTH weight and activation (3 accumulating matmuls, ~bf16 accuracy); fc2
splits only the weights (2 matmuls; single-fp8 gelu output m contributes
~1.2e-2 relative error, within the 2e-2 budget).  Attention S and P@V stay
bf16 (the 2-byte DVE fast path for the mask multiply requires bf16 P).
LayerNorm statistics and residuals stay fp32/f32r.
"""

from contextlib import ExitStack

import numpy as np
import ml_dtypes

import concourse.bass as bass
import concourse.mybir as mybir
import concourse.tile as tile
from concourse import bacc

P = 128
B, N, C = 4, 1024, 1024
H, D = 16, 64
HD = 4 * C
NQ = N // 2          # query tokens per core
NK = N               # key/value tokens per core
CT = C // P          # 8 channel tiles
KB = NK // P         # 8 key-token tiles
SCALE = D ** (-0.5)
EPS = 1e-5
WS = 1024.0          # fp8 weight scale
OS = 64.0            # fp8 scale for attention output o

F32 = mybir.dt.float32
F32R = mybir.dt.float32r
BF16 = mybir.dt.bfloat16
FP8 = mybir.dt.float8e4
AF = mybir.ActivationFunctionType
ALU = mybir.AluOpType
DR = mybir.MatmulPerfMode.DoubleRow
E4NP = ml_dtypes.float8_e4m3


def build_nc(stop_after=None):
    nc = _build_body(stop_after)
    nc.compile()
    return nc


def _build_body(stop_after=None):
    nc = bacc.Bacc("TRN2", target_bir_lowering=False, debug=False, num_devices=8)

    # ---- DRAM I/O (per core) ----
    xT = nc.dram_tensor("xT", [C, NK], F32R, kind="ExternalInput")
    ones_d = nc.dram_tensor("ones_d", [P], F32R, kind="ExternalInput")
    xTqb = nc.dram_tensor("xTqb", [C, NQ], F32, kind="ExternalInput")      # x.T[:, q] + proj_b
    maskT = nc.dram_tensor("maskT", [NK, NQ], BF16, kind="ExternalInput")  # 1-mask, transposed
    qkv_wT = nc.dram_tensor("qkv_wT", [C, 3 * C], FP8, kind="ExternalInput")
    qkvb_qk = nc.dram_tensor("qkvb_qk", [2 * C], F32, kind="ExternalInput")
    qkvb_v = nc.dram_tensor("qkvb_v", [1, C], BF16, kind="ExternalInput")  # x WS
    proj_wT = nc.dram_tensor("proj_wT", [C, C], FP8, kind="ExternalInput")
    fc1a_wT = nc.dram_tensor("fc1a_wT", [C, HD], FP8, kind="ExternalInput")
    fc1r_wT = nc.dram_tensor("fc1r_wT", [C, HD], FP8, kind="ExternalInput")
    fc1b = nc.dram_tensor("fc1b", [HD], F32, kind="ExternalInput")
    fc2a_wT = nc.dram_tensor("fc2a_wT", [HD, C], FP8, kind="ExternalInput")
    fc2r_wT = nc.dram_tensor("fc2r_wT", [HD, C], FP8, kind="ExternalInput")
    fc2b = nc.dram_tensor("fc2b", [C], F32, kind="ExternalInput")
    outT = nc.dram_tensor("outT", [C, NQ], F32, kind="ExternalOutput")

    with tile.TileContext(nc) as tc, ExitStack() as ctx:
        persist = ctx.enter_context(tc.tile_pool(name="persist", bufs=1))
        hpool = ctx.enter_context(tc.tile_pool(name="hpool", bufs=1))
        xstream = ctx.enter_context(tc.tile_pool(name="xstream", bufs=4))
        sqpool = ctx.enter_context(tc.tile_pool(name="sqpool", bufs=2))
        wpool = ctx.enter_context(tc.tile_pool(name="wpool", bufs=3))
        vwpool = ctx.enter_context(tc.tile_pool(name="vwpool", bufs=4))
        wpool2 = ctx.enter_context(tc.tile_pool(name="wpool2", bufs=3))
        wpool3 = ctx.enter_context(tc.tile_pool(name="wpool3", bufs=2))
        w1pool = ctx.enter_context(tc.tile_pool(name="w1pool", bufs=2))
        ptpool = ctx.enter_context(tc.tile_pool(name="ptpool", bufs=4))
        rows2 = ctx.enter_context(tc.tile_pool(name="rows2", bufs=3))
        bcast = ctx.enter_context(tc.tile_pool(name="bcast", bufs=4))
        bcast2 = ctx.enter_context(tc.tile_pool(name="bcast2", bufs=2))
        rbpool = ctx.enter_context(tc.tile_pool(name="rbpool", bufs=2))
        outpool = ctx.enter_context(tc.tile_pool(name="outpool", bufs=2))
        xqpool = ctx.enter_context(tc.tile_pool(name="xqpool", bufs=2))
        h2fpool = ctx.enter_context(tc.tile_pool(name="h2fpool", bufs=2))

        # ---------- constants / biases ----------
        ones_col = persist.tile([P, 1], F32R, tag="ones_col")
        nc.sync.dma_start(ones_col, ones_d.rearrange("(p o) -> p o", o=1))
        ones_k1 = persist.tile([1, P], BF16, tag="ones_k1")
        nc.gpsimd.dma_start(ones_k1, ones_d.rearrange("(o p) -> o p", o=1))
        epst = persist.tile([1, 1], F32, tag="eps")
        nc.vector.memset(epst, EPS)
        qkb_sb = persist.tile([P, 16], F32, tag="qkb")
        nc.sync.dma_start(qkb_sb, qkvb_qk.rearrange("(o p) -> p o", p=P))
        fc1b_sb = persist.tile([P, 32], F32, tag="fc1b")
        nc.sync.dma_start(fc1b_sb, fc1b.rearrange("(o p) -> p o", p=P))
        fc2b_sb = persist.tile([P, 8], F32, tag="fc2b")
        nc.sync.dma_start(fc2b_sb, fc2b.rearrange("(o p) -> p o", p=P))
        vb_row = persist.tile([1, C], BF16, tag="vb")
        nc.sync.dma_start(vb_row, qkvb_v.ap())

        # ---------- phases 1+2: LN1, pipelined per token-half ----------
        # hT is split into two tiles (one per 512-token half) so the QKV
        # matmuls for half 0 can start while half 1 is still normalizing.
        xTr = xT.rearrange("(ct p) n -> p ct n", p=P)
        hTh = []
        psA_ctx = ExitStack()
        psA = psA_ctx.enter_context(tc.tile_pool(name="psA", bufs=2, space="PSUM"))
        for nh in range(2):
            sl = slice(nh * 512, (nh + 1) * 512)
            ps_s1 = psA.tile([1, 512], F32, tag="s", name=f"ps_s1_{nh}")
            ps_s2 = psA.tile([1, 512], F32, tag="s2", name=f"ps_s2_{nh}")
            for t in range(CT // 2):
                xc = xstream.tile([P, 2, 512], F32R, tag="xc", name=f"xc_{nh}_{t}")
                nc.sync.dma_start(xc, xTr[:, 2 * t:2 * t + 2, sl])
                for i in range(2):
                    ct = 2 * t + i
                    sq = sqpool.tile([P, 512], F32R, tag="sq", name=f"sq_{nh}_{ct}")
                    nc.gpsimd.tensor_tensor(sq, xc[:, i, :].bitcast(F32),
                                            xc[:, i, :].bitcast(F32), ALU.mult)
                    nc.tensor.matmul(ps_s1, ones_col, xc[:, i, :],
                                     start=(ct == 0), stop=(ct == CT - 1))
                    nc.tensor.matmul(ps_s2, ones_col, sq,
                                     start=(ct == 0), stop=(ct == CT - 1))
            mu_row = rows2.tile([1, 512], F32, tag="r512ln", name=f"mu_{nh}")
            nc.vector.tensor_scalar_mul(mu_row, ps_s1, 1.0 / C)
            e2_row = rows2.tile([1, 512], F32, tag="r512ln", name=f"e2_{nh}")
            nc.vector.tensor_scalar_mul(e2_row, ps_s2, 1.0 / C)
            tmp_row = rows2.tile([1, 512], F32, tag="r512ln", name=f"tmp_{nh}")
            nc.vector.tensor_tensor(tmp_row, mu_row, mu_row, ALU.mult)
            nc.vector.tensor_tensor(e2_row, e2_row, tmp_row, ALU.subtract)
            nc.scalar.activation(e2_row, e2_row, AF.Sqrt, bias=epst[:, :])
            nc.vector.reciprocal(e2_row, e2_row)
            mu_bc = bcast.tile([P, 512], F32, tag="b512", name=f"mu_b_{nh}")
            nc.gpsimd.partition_broadcast(mu_bc[:, :], mu_row[:, :])
            rs_bc = bcast.tile([P, 512], F32, tag="b512", name=f"rs_b_{nh}")
            nc.gpsimd.partition_broadcast(rs_bc[:, :], e2_row[:, :])
            hT = hpool.tile([P, CT, 512], FP8, tag=f"h{nh}")
            tmpf = hpool.tile([P, 512], F32, tag=f"htmp{nh}")
            for t in range(CT // 2):
                xc = xstream.tile([P, 2, 512], F32R, tag="xc", name=f"xc2_{nh}_{t}")
                nc.sync.dma_start(xc, xTr[:, 2 * t:2 * t + 2, sl])
                for i in range(2):
                    ct = 2 * t + i
                    nc.vector.tensor_tensor(tmpf, xc[:, i, :].bitcast(F32), mu_bc,
                                            ALU.subtract)
                    nc.vector.tensor_tensor(hT[:, ct, :], tmpf, rs_bc, ALU.mult)
            hTh.append(hT)
        psA_ctx.close()

        if stop_after == 'ln1':
            return nc
        # ---------- phase 3: q^T, k^T (fp8 DoubleRow matmuls) ----------
        # Host passes xT ROLLED per core: the query half is always tokens
        # [0:512] (= hTh[0]); K/V cover both halves with mask columns matched.
        qT = persist.tile([P, CT, NQ], BF16, tag="qT")
        kT = persist.tile([P, CT, NK], BF16, tag="kT")
        wT = qkv_wT.rearrange("(ct p) m -> p ct m", p=P)
        psB_ctx = ExitStack()
        psB = psB_ctx.enter_context(tc.tile_pool(name="psB", bufs=4, space="PSUM"))
        for wg in range(4):          # 4 groups of 4 chan-blocks (q,q,k,k)
            is_q = wg < 2
            nhs = [0] if is_q else [0, 1]
            for nh in nhs:
                tsl = slice(nh * 512, (nh + 1) * 512)
                pss = [psB.tile([P, 512], F32, tag="mm", name=f"ps_qk{wg}_{nh}_{j}")
                       for j in range(4)]
                for t in range(CT // 2):
                    wt = wpool.tile([P, 2, 512], FP8, tag="w")
                    nc.sync.dma_start(wt, wT[:, 2 * t:2 * t + 2, wg * 512:(wg + 1) * 512])
                    for j in range(4):
                        nc.tensor.matmul(pss[j], wt[:, :, j * P:(j + 1) * P],
                                         hTh[nh][:, 2 * t:2 * t + 2, :],
                                         start=(t == 0), stop=(t == CT // 2 - 1),
                                         perf_mode=DR)
                for j in range(4):
                    mb = wg * 4 + j
                    bias = qkb_sb[:, mb:mb + 1]
                    if is_q:
                        nc.scalar.activation(qT[:, mb, :], pss[j], AF.Identity,
                                             bias=bias, scale=1.0 / WS)
                    else:
                        nc.scalar.activation(kT[:, mb - 8, tsl], pss[j], AF.Identity,
                                             bias=bias, scale=1.0 / WS)

        # mask (bf16, [k, q] as [128, kb, q]) - needed from phase 5 on; DMA'd
        # here so it doesn't delay the LN1 x stream or the qkv weights.
        mask_sb = persist.tile([P, KB, NQ], BF16, tag="mask")
        mT = maskT.rearrange("(kb p) q -> p kb q", p=P)
        nc.sync.dma_start(mask_sb, mT)

        if stop_after == 'qk':
            psB_ctx.close()
            return nc
        # ---------- phase 4: v token-major + ones column (fp8 DR) ----------
        # V_aug: [128 tok, kb, head, 65] (65th col = 1.0 for denominator)
        V_aug = persist.tile([P, KB, H, 65], BF16, tag="vaug")
        nc.vector.memset(V_aug[:, :, :, 64:65], 1.0)
        psV_ctx = ExitStack()
        psV = psV_ctx.enter_context(tc.tile_pool(name="psV", bufs=4, space="PSUM"))
        for nh in range(4):
            wvs = []
            for t in range(CT // 2):
                wv = vwpool.tile([P, 2, 256], FP8, tag="vw", name=f"vw_{nh}_{t}")
                nc.sync.dma_start(wv, wT[:, 2 * t:2 * t + 2,
                                         2048 + nh * 256: 2048 + (nh + 1) * 256])
                wvs.append(wv)
            for tb in range(KB):
                psv = psV.tile([P, 256], F32, tag="mmv", name=f"psv_{nh}_{tb}")
                for t in range(CT // 2):
                    hsrc = hTh[tb // 4][:, 2 * t:2 * t + 2,
                                        (tb % 4) * P:(tb % 4 + 1) * P]
                    nc.tensor.matmul(psv, hsrc, wvs[t],
                                     start=(t == 0), stop=False, perf_mode=DR)
                nc.tensor.matmul(psv, ones_k1,
                                 vb_row[:, nh * 256:(nh + 1) * 256],
                                 start=False, stop=True)
                nc.scalar.activation(
                    V_aug[:, tb, nh * 4:(nh + 1) * 4, 0:64],
                    psv.rearrange("p (h d) -> p h d", d=64),
                    AF.Copy, scale=1.0 / WS)
        psV_ctx.close()
        psB_ctx.close()

        if stop_after == 'v':
            return nc
        # ---------- phase 5: attention per head (bf16 S / PV) ----------
        psS_ctx = ExitStack()
        psS = psS_ctx.enter_context(tc.tile_pool(name="psS", bufs=2, space="PSUM"))
        psO = psS_ctx.enter_context(tc.tile_pool(name="psO", bufs=2, space="PSUM"))
        oT = persist.tile([P, CT, NQ], FP8, tag="oT")
        for h in range(H):
            j, base = h // 2, (h % 2) * 64
            po = psO.tile([65, NQ], F32, tag="o")
            for g in range(4):
                psg = psS.tile([P, 2, 512], F32, tag="sg", name=f"psg_{h}_{g}")
                for i in range(2):
                    kb = 2 * g + i
                    nc.tensor.matmul(psg[:, i, :],
                                     kT[base:base + 64, j, kb * P:(kb + 1) * P],
                                     qT[base:base + 64, j, :], start=True, stop=True)
                PT = ptpool.tile([P, 2, NQ], BF16, tag="pt", name=f"pt_{h}_{g}")
                nc.scalar.activation(PT, psg, AF.Exp)
                nc.vector.tensor_tensor(PT, PT, mask_sb[:, 2 * g:2 * g + 2, :], ALU.mult)
                for i in range(2):
                    kb = 2 * g + i
                    nc.tensor.matmul(po, V_aug[:, kb, h, :], PT[:, i, :],
                                     start=(kb == 0), stop=(kb == KB - 1))
            # oT = fp8(OS * num / (1 + den));  drow = (den + 1) / OS
            drow = rows2.tile([1, NQ], F32, tag="r512")
            nc.vector.tensor_scalar(drow, po[64:65, :], 1.0, 1.0 / OS,
                                    ALU.add, ALU.mult)
            nc.vector.reciprocal(drow, drow)
            rb = rbpool.tile([64, NQ], F32, tag="rb")
            nc.gpsimd.partition_broadcast(rb[:, :], drow[:, :])
            nc.vector.tensor_tensor(oT[base:base + 64, j, :], po[0:64, :], rb, ALU.mult)
        psS_ctx.close()

        if stop_after == 'attn':
            return nc
        # ---------- phase 6: proj (fp8 DR) + residual -> x2T; LN2 stats ----------
        psB2_ctx = ExitStack()
        psB2 = psB2_ctx.enter_context(tc.tile_pool(name="psB2", bufs=4, space="PSUM"))
        psC_ctx = ExitStack()
        psC = psC_ctx.enter_context(tc.tile_pool(name="psC", bufs=2, space="PSUM"))
        ps_t1 = psC.tile([1, NQ], F32, tag="s2")
        ps_t2 = psC.tile([1, NQ], F32, tag="s2")
        x2T = persist.tile([P, CT, NQ], F32R, tag="x2T")
        pTr = proj_wT.rearrange("(ct p) m -> p ct m", p=P)
        for ob in range(CT):
            wp = wpool3.tile([P, CT, P], FP8, tag="wp")
            nc.sync.dma_start(wp, pTr[:, :, ob * P:(ob + 1) * P])
            psp = psB2.tile([P, 512], F32, tag="mm")
            for t in range(CT // 2):
                nc.tensor.matmul(psp, wp[:, 2 * t:2 * t + 2, :],
                                 oT[:, 2 * t:2 * t + 2, :],
                                 start=(t == 0), stop=(t == CT // 2 - 1),
                                 perf_mode=DR)
            xq = xqpool.tile([P, NQ], F32, tag="xq")
            nc.sync.dma_start(xq, xTqb[ob * P:(ob + 1) * P, :])
            nc.vector.scalar_tensor_tensor(x2T[:, ob, :], psp, 1.0 / (WS * OS), xq,
                                           ALU.mult, ALU.add)
            # LN2 stats as soon as each channel block is ready
            sq2 = sqpool.tile([P, 512], F32R, tag="sq", name=f"sq2_{ob}")
            nc.gpsimd.tensor_tensor(sq2, x2T[:, ob, :].bitcast(F32),
                                    x2T[:, ob, :].bitcast(F32), ALU.mult)
            nc.tensor.matmul(ps_t1, ones_col, x2T[:, ob, :],
                             start=(ob == 0), stop=(ob == CT - 1))
            nc.tensor.matmul(ps_t2, ones_col, sq2,
                             start=(ob == 0), stop=(ob == CT - 1))

        if stop_after == 'proj':
            psC_ctx.close()
            psB2_ctx.close()
            return nc
        # ---------- phase 7: LN2 -> h2a/h2r (two-term fp8 split) ----------
        mu2 = rows2.tile([1, NQ], F32R, tag="r512ln")
        nc.vector.tensor_scalar_mul(mu2, ps_t1, 1.0 / C)
        e22 = rows2.tile([1, NQ], F32R, tag="r512ln")
        nc.vector.tensor_scalar_mul(e22, ps_t2, 1.0 / C)
        tmp2 = rows2.tile([1, NQ], F32R, tag="r512ln")
        nc.vector.tensor_tensor(tmp2, mu2.bitcast(F32), mu2.bitcast(F32), ALU.mult)
        nc.vector.tensor_tensor(e22, e22.bitcast(F32), tmp2.bitcast(F32), ALU.subtract)
        nc.scalar.activation(e22, e22.bitcast(F32), AF.Sqrt, bias=epst[:, :])
        with nc.allow_low_precision(reason="f32r rsig rounding ~1e-4 is fine"):
            nc.vector.reciprocal(e22, e22.bitcast(F32))
        psC_ctx.close()
        mu2b = bcast2.tile([P, NQ], F32, tag="bc2")
        nc.gpsimd.partition_broadcast(mu2b[:, :], mu2.bitcast(F32)[:, :])
        rs2b = bcast2.tile([P, NQ], F32, tag="bc2")
        nc.gpsimd.partition_broadcast(rs2b[:, :], e22.bitcast(F32)[:, :])
        h2aT = persist.tile([P, CT, NQ], FP8, tag="h2a")
        h2rT = persist.tile([P, CT, NQ], FP8, tag="h2r")
        for ob in range(CT):
            h2tmp = h2fpool.tile([P, 512], F32, tag="h2t", name=f"h2t_{ob}")
            nc.vector.tensor_tensor(h2tmp, x2T[:, ob, :].bitcast(F32), mu2b,
                                    ALU.subtract)
            h2f = h2fpool.tile([P, 512], F32, tag="h2f", name=f"h2f_{ob}")
            nc.vector.tensor_tensor(h2f, h2tmp, rs2b, ALU.mult)
            nc.scalar.activation(h2aT[:, ob, :], h2f, AF.Copy)
            nc.vector.tensor_tensor(h2rT[:, ob, :], h2f, h2aT[:, ob, :], ALU.subtract)

        if stop_after == 'ln2':
            psB2_ctx.close()
            return nc
        # ---------- phase 8a: fc1 (3-term fp8 DR) + gelu -> m8 (SBUF) ----------
        m8 = persist.tile([P, 32, NQ], FP8, tag="m8")
        w1aT = fc1a_wT.rearrange("(ct p) m -> p ct m", p=P)
        w1rT = fc1r_wT.rearrange("(ct p) m -> p ct m", p=P)
        psB2_ctx.close()
        psF_ctx = ExitStack()
        psF = psF_ctx.enter_context(tc.tile_pool(name="psF", bufs=8, space="PSUM"))
        for hg in range(8):
            pss = [psF.tile([P, 512], F32, tag="mm", name=f"ps_fc1_{hg}_{j}")
                   for j in range(4)]
            w1a = w1pool.tile([P, CT, 512], FP8, tag="w1a")
            nc.sync.dma_start(w1a, w1aT[:, :, hg * 512:(hg + 1) * 512])
            w1r = w1pool.tile([P, CT, 512], FP8, tag="w1r")
            nc.sync.dma_start(w1r, w1rT[:, :, hg * 512:(hg + 1) * 512])
            for term, (wt_, ht_) in enumerate(
                    [(w1a, h2aT), (w1a, h2rT), (w1r, h2aT)]):
                for t in range(CT // 2):
                    for j in range(4):
                        nc.tensor.matmul(pss[j],
                                         wt_[:, 2 * t:2 * t + 2, j * P:(j + 1) * P],
                                         ht_[:, 2 * t:2 * t + 2, :],
                                         start=(term == 0 and t == 0),
                                         stop=(term == 2 and t == CT // 2 - 1),
                                         perf_mode=DR)
            for j in range(4):
                hb = hg * 4 + j
                nc.scalar.activation(m8[:, hb, :], pss[j], AF.Gelu,
                                     bias=fc1b_sb[:, hb:hb + 1], scale=1.0 / WS)
        psF_ctx.close()

        if stop_after == 'fc1':
            return nc
        # ---------- phase 8b: fc2 (2-term fp8 DR) + bias + residual ----------
        # two column groups so the first half's output drains while the
        # second half still does matmuls
        psD_ctx = ExitStack()
        psD = psD_ctx.enter_context(tc.tile_pool(name="psD", bufs=8, space="PSUM"))
        w2aT = fc2a_wT.rearrange("(ht p) m -> p ht m", p=P)
        w2rT = fc2r_wT.rearrange("(ht p) m -> p ht m", p=P)
        for grp in range(2):
            csl = slice(grp * 512, (grp + 1) * 512)
            ps_oc = [psD.tile([P, 512], F32, tag="fc2", name=f"ps_fc2_{grp}_{j}")
                     for j in range(4)]
            for tp in range(16):
                w2a = wpool2.tile([P, 2, 512], FP8, tag="w2a")
                nc.sync.dma_start(w2a, w2aT[:, 2 * tp:2 * tp + 2, csl])
                w2r = wpool2.tile([P, 2, 512], FP8, tag="w2r")
                nc.sync.dma_start(w2r, w2rT[:, 2 * tp:2 * tp + 2, csl])
                for j in range(4):
                    nc.tensor.matmul(ps_oc[j], w2a[:, :, j * P:(j + 1) * P],
                                     m8[:, 2 * tp:2 * tp + 2, :],
                                     start=(tp == 0), stop=False, perf_mode=DR)
                    nc.tensor.matmul(ps_oc[j], w2r[:, :, j * P:(j + 1) * P],
                                     m8[:, 2 * tp:2 * tp + 2, :],
                                     start=False, stop=(tp == 15), perf_mode=DR)
            for j in range(4):
                ob = grp * 4 + j
                ot = outpool.tile([P, NQ], F32, tag="out")
                nc.vector.tensor_scalar(ot, ps_oc[j], 1.0 / WS,
                                        fc2b_sb[:, ob:ob + 1], ALU.mult, ALU.add)
                nc.vector.tensor_tensor(ot, ot, x2T[:, ob, :].bitcast(F32), ALU.add)
                nc.sync.dma_start(outT[ob * P:(ob + 1) * P, :], ot)
        psD_ctx.close()

    return nc


# ---------------------------------------------------------------------------
# Host side: shard, run, gather
# ---------------------------------------------------------------------------
_RUNNER = None


class _Runner:
    """Minimal SPMD executor via bass2jax custom call (axon PJRT path)."""

    def __init__(self, nc, n_cores):
        import jax
        from jax.sharding import Mesh, PartitionSpec
        from jax.experimental.shard_map import shard_map
        from concourse.bass2jax import (_bass_exec_p, install_neuronx_cc_hook,
                                        partition_id_tensor)
        install_neuronx_cc_hook()
        self.jax = jax
        self.nc = nc
        self.n_cores = n_cores
        partition_name = nc.partition_id_tensor.name if nc.partition_id_tensor else None
        in_names, out_names, out_avals, zero_outs = [], [], [], []
        for alloc in nc.m.functions[0].allocations:
            if not isinstance(alloc, mybir.MemoryLocationSet):
                continue
            name = alloc.memorylocations[0].name
            if alloc.kind == "ExternalInput":
                if name != partition_name:
                    in_names.append(name)
            elif alloc.kind == "ExternalOutput":
                shape = tuple(alloc.tensor_shape)
                dtype = mybir.dt.np(alloc.dtype)
                out_names.append(name)
                out_avals.append(jax.core.ShapedArray(shape, dtype))
                zero_outs.append(np.zeros(shape, dtype))
        self.in_names, self.out_names = in_names, out_names
        self._out_avals, self._zero_outs = out_avals, zero_outs
        n_params = len(in_names)
        all_in = in_names + out_names + ([partition_name] if partition_name else [])

        def _body(*args):
            operands = list(args)
            if partition_name is not None:
                operands.append(partition_id_tensor())
            return tuple(_bass_exec_p.bind(
                *operands, out_avals=tuple(out_avals), in_names=tuple(all_in),
                out_names=tuple(out_names), lowering_input_output_aliases=(),
                sim_require_finite=True, sim_require_nnan=True, nc=nc))

        devices = jax.devices()[:n_cores]
        mesh = Mesh(np.asarray(devices), ("core",))
        nspec = n_params + len(out_names)
        self._fn = jax.jit(
            shard_map(_body, mesh=mesh, in_specs=(PartitionSpec("core"),) * nspec,
                      out_specs=(PartitionSpec("core"),) * len(out_names),
                      check_rep=False),
            keep_unused=True)

    def run(self, in_maps):
        n = self.n_cores
        per_core = [[np.ascontiguousarray(m[k]) for k in self.in_names] for m in in_maps]
        args = [np.concatenate([per_core[c][i] for c in range(n)], axis=0)
                for i in range(len(self.in_names))]
        args += [np.zeros((n * z.shape[0], *z.shape[1:]), z.dtype) for z in self._zero_outs]
        outs = self._fn(*args)
        self.jax.block_until_ready(outs)
        return [
            {name: np.asarray(outs[i]).reshape(n, *self._out_avals[i].shape)[c]
             for i, name in enumerate(self.out_names)}
            for c in range(n)
        ]


def _get_runner():
    global _RUNNER
    if _RUNNER is None:
        _RUNNER = _Runner(build_nc(), 8)
    return _RUNNER


def _split_e4m3(w):
    """two-term e4m3 split of an fp32 array (already scaled)."""
    hi = w.astype(E4NP)
    lo = (w - hi.astype(np.float32)).astype(E4NP)
    return hi, lo


def kernel(x, mask, ln1_g, ln1_b, qkv_w, qkv_b, proj_w, proj_b,
           ln2_g, ln2_b, fc1_w, fc1_b, fc2_w, fc2_b):
    x = np.asarray(x, np.float32)
    mask = np.asarray(mask, bool)
    ln1_g = np.asarray(ln1_g, np.float32); ln1_b = np.asarray(ln1_b, np.float32)
    qkv_w = np.asarray(qkv_w, np.float32); qkv_b = np.asarray(qkv_b, np.float32)
    proj_w = np.asarray(proj_w, np.float32); proj_b = np.asarray(proj_b, np.float32)
    ln2_g = np.asarray(ln2_g, np.float32); ln2_b = np.asarray(ln2_b, np.float32)
    fc1_w = np.asarray(fc1_w, np.float32); fc1_b = np.asarray(fc1_b, np.float32)
    fc2_w = np.asarray(fc2_w, np.float32); fc2_b = np.asarray(fc2_b, np.float32)

    # fold LN1 gain/bias + softmax scale into qkv weights/bias
    Wq = qkv_w * ln1_g[None, :]
    qb = qkv_w @ ln1_b + qkv_b
    Wq[:C] *= SCALE
    qb = qb.copy(); qb[:C] *= SCALE
    qkv_wT = np.ascontiguousarray(Wq.T * WS).astype(E4NP)
    # fold LN2 gain/bias into fc1
    W1 = fc1_w * ln2_g[None, :]
    fb1 = fc1_w @ ln2_b + fc1_b
    fc1a_wT, fc1r_wT = _split_e4m3(np.ascontiguousarray(W1.T * WS))
    proj_wT = np.ascontiguousarray(proj_w.T * WS).astype(E4NP)
    fc2a_wT, fc2r_wT = _split_e4m3(np.ascontiguousarray(fc2_w.T * WS))
    qkvb_v = (np.ascontiguousarray(qb[2 * C:].reshape(1, C)) * WS).astype(ml_dtypes.bfloat16)
    maskmul = (~mask).astype(ml_dtypes.bfloat16)

    in_maps = []
    for core in range(8):
        b, s = core // 2, core % 2
        # roll x so this core's query half is always tokens [0:512]
        xb = x[b]                         # [N, C]
        xroll = np.roll(xb, -s * NQ, axis=0)
        xT_c = np.ascontiguousarray(xroll.T)               # [C, NK]
        xTqb_c = np.ascontiguousarray(xb[s * NQ:(s + 1) * NQ].T) + proj_b[:, None]
        # mask rows q = this core's queries; key order must match rolled order
        mrow = maskmul[b, s * NQ:(s + 1) * NQ]             # [NQ, N] keys orig order
        mroll = np.roll(mrow, -s * NQ, axis=1)             # keys in rolled order
        maskT_c = np.ascontiguousarray(mroll.T)            # [NK, NQ]
        in_maps.append({
            "ones_d": np.ones(P, np.float32),
            "xT": xT_c.astype(np.float32),
            "xTqb": xTqb_c.astype(np.float32),
            "maskT": maskT_c,
            "qkv_wT": qkv_wT, "qkvb_qk": qb[:2 * C].copy(), "qkvb_v": qkvb_v,
            "proj_wT": proj_wT,
            "fc1a_wT": fc1a_wT, "fc1r_wT": fc1r_wT, "fc1b": fb1.copy(),
            "fc2a_wT": fc2a_wT, "fc2r_wT": fc2r_wT, "fc2b": fc2_b.copy(),
        })

    results = _get_runner().run(in_maps)
    out = np.empty((B, N, C), np.float32)
    for core in range(8):
        b, s = core // 2, core % 2
        out[b, s * NQ:(s + 1) * NQ, :] = results[core]["outT"].T
    return out
